# revision 8
# baseline (speedup 1.0000x reference)
"""Bass/Trainium2 kernel for nn_Blob_DC_and_BCE_loss (loss_fn).

Strategy (v2)
-------------
The loss decomposes into per-sample global sums plus small ROI-box
corrections.  Only three global per-voxel quantities are needed:

    A_s = sum softplus(x)   (for BCE)
    P_s = sum sigmoid(x)    (for dice denominators)
    Sx_s = sum x

because every y=1 voxel lives inside a target component, which is fully
covered by the ROI boxes: sum x*y, sum p*y and sum y are recovered from
per-box masked sums plus host-side integer counts.

Identities used on device (one Exp+Ln activation table, no reloads):
    u = exp(-x)            ACT
    l = ln(1+u)            ACT  (= softplus(-x); accum -> sum per sample)
    p = reciprocal(1+u)    DVE  (= sigmoid(x); PE colsums -> sum per sample)
    A_s = sum l + sum x    (sum x via PE matmul colsums, free elsewhere)
    P_s = sum p

ROI boxes (one 32^3 box per core): host builds 0/1 masks
    o  = owned, oy = owned&y, o1y = owned&(1-y), g = owned&t==0&m==0
and the device computes eight masked sums with single-instruction
scalar_tensor_tensor multiply+accumulate, split across DVE and Pool.

All DMA-touched tensors are bf16 (x, masks); all on-chip intermediates
are fp32 (chained bf16 rounding of ln(1+u) biases A by ~1%, far above
budget).  I/O is batched: 6 input DMA chunks, one [128,14] fp32 output
DMA per core (the v1 kernel's 42 tiny DMAs serialized ~27us on the SP
sequencer + HWDGE).

Host: CC labeling, box/ownership setup, integer mask counts, O(1)
scalar assembly.  Device: all O(N) float math.
"""

import math
import os

import numpy as np

try:
    import ml_dtypes

    BF16 = ml_dtypes.bfloat16
except Exception:  # pragma: no cover
    BF16 = None

B = 2
D = H = W = 128
N = D * H * W
NCORES = 8
SLAB = D // NCORES            # 16 depth slices per core
GS = SLAB * H * W // 128      # 2048: free-dim cols of one sample slab
BOX = 32                      # ROI box edge
BC = BOX ** 3 // 128          # 256: free-dim cols of one box
GTOT = B * GS + BC            # 4352 cols in the x input tensor
MCOLS = 4 * BC                # 1024 cols in the mask tensor (o|oy|o1y|g)
K_DEV = 4                     # labels per sample handled in fast path
LOG2 = math.log(2.0)
SMOOTH = 1e-5
NSLAB = 128 * GS              # voxels per core per sample

# output columns of the per-core [128, OC] accumulator.
# cols 0-4: direct accum_out writes (ACT ln accums + DVE STT msum accums);
# cols 5-13: PE-colsum results copied from one [128,9] PSUM tile.
C_L0, C_L1 = 0, 1             # sum ln(1+u) per sample (ACT accum)
C_PO, C_POY, C_PG = 2, 3, 4   # box sigmoid(x) masked sums (DVE STT accum)
C_X0, C_X1 = 5, 6             # sum x per sample (PE colsum)
C_P0, C_P1 = 7, 8             # sum sigmoid(x) per sample (PE colsum)
C_LO, C_LG = 9, 10            # box ln masked sums (Pool TT + PE colsum)
C_X1Y, C_XG, C_XOY = 11, 12, 13  # box x masked sums (Pool TT + PE colsum)
OC = 14
PS0 = C_X0                    # psum column c maps to acc column PS0 + c


# --------------------------------------------------------------------------
# host-side connected components (scipy if present, numpy fallback)
# --------------------------------------------------------------------------

def _label_np(mask):
    """6-connectivity CC labeling, pure numpy (iterative min-propagation)."""
    lab = np.where(mask, np.arange(1, mask.size + 1, dtype=np.int64
                                   ).reshape(mask.shape), 0)
    while True:
        new = lab.copy()
        sl = new[1:, :, :]; np.minimum(sl, np.where(lab[:-1] > 0, lab[:-1], sl), out=sl)
        sl = new[:-1, :, :]; np.minimum(sl, np.where(lab[1:] > 0, lab[1:], sl), out=sl)
        sl = new[:, 1:, :]; np.minimum(sl, np.where(lab[:, :-1] > 0, lab[:, :-1], sl), out=sl)
        sl = new[:, :-1, :]; np.minimum(sl, np.where(lab[:, 1:] > 0, lab[:, 1:], sl), out=sl)
        sl = new[:, :, 1:]; np.minimum(sl, np.where(lab[:, :, :-1] > 0, lab[:, :, :-1], sl), out=sl)
        sl = new[:, :, :-1]; np.minimum(sl, np.where(lab[:, :, 1:] > 0, lab[:, :, 1:], sl), out=sl)
        new = np.where(mask, new, 0)
        if np.array_equal(new, lab):
            break
        lab = new
    uniq = np.unique(lab[lab > 0])
    remap = np.zeros(int(lab.max()) + 1, np.int64)
    remap[uniq] = np.arange(1, len(uniq) + 1)
    return remap[lab], len(uniq)


def _cc_label(mask):
    try:
        from scipy import ndimage as ndi
        st = ndi.generate_binary_structure(3, 1)
        lab, n = ndi.label(mask, structure=st)
        return lab.astype(np.int64), int(n)
    except Exception:
        return _label_np(mask)


CROP_MARGIN = 24   # predicted comps matched to a target stay well inside this


def _host_metadata(x, y):
    """Per-sample rank volumes t8/m8 and component counts.

    All labeling runs on a crop = target bounding box + CROP_MARGIN.  A
    predicted component can only be matched to a target if it intersects
    it, and matched components are small appendages of the targets, so
    anything outside the crop has t = m = 0.  The crop assumption is
    verified (no predicted foreground on the crop faces is labeled).
    """
    meta = []
    for b in range(B):
        tgt_full = y[b, 0] > 0.5
        pred_full = x[b, 0] >= 0.0
        if not tgt_full.any():
            meta.append(dict(t8=np.zeros((D, H, W), np.float32),
                             m8=np.zeros((D, H, W), np.float32), n_cc=0))
            continue
        idx = np.argwhere(tgt_full)
        lo = np.maximum(idx.min(axis=0) - CROP_MARGIN, 0)
        hi = np.minimum(idx.max(axis=0) + 1 + CROP_MARGIN, (D, H, W))
        sl = tuple(slice(int(a), int(c)) for a, c in zip(lo, hi))
        tgt = tgt_full[sl]
        pred = pred_full[sl]
        lin1 = (np.arange(N, dtype=np.int64).reshape(D, H, W)[sl] + 1)
        tlab, ntc = _cc_label(tgt)
        plab, npc = _cc_label(pred)
        # reference label value = max linear index + 1 within target comp
        tmax = np.zeros(ntc + 1, np.int64)
        np.maximum.at(tmax, tlab.ravel(), np.where(tgt, lin1, 0).ravel())
        tval = np.where(tgt, tmax[tlab], 0)
        # map each predicted comp to the max target label it overlaps
        pmax = np.zeros(npc + 1, np.int64)
        np.maximum.at(pmax, plab.ravel(), tval.ravel())
        mval = np.where(pred, pmax[plab], 0)
        # crop-validity: no matched predicted voxel may touch a crop face
        ok = True
        for ax in range(3):
            for face in (0, -1):
                f = [slice(None)] * 3
                f[ax] = face
                if (mval[tuple(f)] > 0).any():
                    ok = False
        # ranks: descending reference label order (top_k order)
        labels_desc = np.sort(np.unique(tval[tval > 0]))[::-1]
        n_cc = len(labels_desc)
        rank_of = np.zeros(int(tval.max()) + 1 if n_cc else 1, np.int64)
        for i, L in enumerate(labels_desc):
            rank_of[L] = i + 1
        t8 = np.zeros((D, H, W), np.float32)
        m8 = np.zeros((D, H, W), np.float32)
        t8[sl] = rank_of[tval]
        m8[sl] = rank_of[mval]
        meta.append(dict(t8=t8, m8=m8, n_cc=n_cc, crop_ok=ok))
    return meta


def _build_boxes(meta):
    """Cover the interesting voxels with <= NCORES boxes of BOX^3.

    Each connected cluster of the interesting set (target comp + its
    matched predicted comps) is covered by a grid of boxes over its bbox.
    Returns list of (sample, d0, h0, w0) and per-sample ownership arrays
    (box index owning each voxel, -1 if none).  Returns (None, None) when
    more than NCORES boxes would be needed (general fallback).
    """
    boxes = []
    owners = []
    for b in range(B):
        t8, m8 = meta[b]["t8"], meta[b]["m8"]
        interesting = (t8 > 0) | (m8 > 0)
        own = np.full((D, H, W), -1, np.int32)
        owners.append(own)
        if not interesting.any():
            continue
        clab, ncl = _cc_label(interesting)
        sample_boxes = []
        for ci in range(1, ncl + 1):
            idx = np.argwhere(clab == ci)
            lo, hi = idx.min(axis=0), idx.max(axis=0)  # inclusive
            starts_per_dim = []
            for ax in range(3):
                ext = int(hi[ax] - lo[ax] + 1)
                nb = (ext + BOX - 1) // BOX
                if nb == 1:
                    s0 = int(lo[ax]) - (BOX - ext) // 2
                    starts_per_dim.append([min(max(s0, 0), D - BOX)])
                else:
                    step = (ext - BOX) / (nb - 1)
                    starts_per_dim.append(
                        [min(max(int(lo[ax] + round(i * step)), 0), D - BOX)
                         for i in range(nb)])
            for sd in starts_per_dim[0]:
                for sh in starts_per_dim[1]:
                    for sw in starts_per_dim[2]:
                        bi = len(boxes)
                        if bi >= NCORES:
                            return None, None
                        boxes.append((b, sd, sh, sw))
                        sample_boxes.append((bi, ci, sd, sh, sw))
                        # interesting voxels of THIS cluster claim the box
                        sl = (slice(sd, sd + BOX), slice(sh, sh + BOX),
                              slice(sw, sw + BOX))
                        region = own[sl]
                        region[(clab[sl] == ci) & (region < 0)] = bi
        # background (non-interesting) voxels: first covering box wins
        for bi, ci, sd, sh, sw in sample_boxes:
            sl = (slice(sd, sd + BOX), slice(sh, sh + BOX),
                  slice(sw, sw + BOX))
            region = own[sl]
            region[region < 0] = bi
    for b in range(B):
        t8, m8 = meta[b]["t8"], meta[b]["m8"]
        if (((t8 > 0) | (m8 > 0)) & (owners[b] < 0)).any():
            return None, None
    return boxes, owners


def _box_ranks(meta, boxes, owners):
    """Per box: set of component ranks present among its owned voxels."""
    ranks = []
    for i, (bsmp, bd, bh, bw) in enumerate(boxes):
        sl = (slice(bd, bd + BOX), slice(bh, bh + BOX), slice(bw, bw + BOX))
        owned = owners[bsmp][sl] == i
        t = meta[bsmp]["t8"][sl][owned]
        m = meta[bsmp]["m8"][sl][owned]
        rs = set(np.unique(t[t > 0]).tolist()) | set(np.unique(m[m > 0]).tolist())
        ranks.append({int(r) for r in rs})
    return ranks


def _box_masks_counts(x, y, meta, boxes, owners):
    """Per box: the four bf16 0/1 mask planes + integer counts."""
    out = []
    for i, (bs, bd, bh, bw) in enumerate(boxes):
        sl = (slice(bd, bd + BOX), slice(bh, bh + BOX), slice(bw, bw + BOX))
        owned = owners[bs][sl] == i
        t8 = meta[bs]["t8"][sl]
        m8 = meta[bs]["m8"][sl]
        yb = y[bs, 0][sl] > 0.5
        o = owned
        oy = owned & yb
        o1y = owned & ~yb
        g = owned & (t8 == 0) & (m8 == 0)
        out.append(dict(
            o=o, oy=oy, o1y=o1y, g=g,
            cnt_o=float(o.sum()), cnt_oy=float(oy.sum()),
            cnt_g=float(g.sum()),
            xb=x[bs, 0][sl],
        ))
    return out


def _build_in_maps(x, y, meta, boxes, owners):
    """Per-core input tensors: gx [128, GTOT] bf16, mk [128, MCOLS] bf16."""
    bm = _box_masks_counts(x, y, meta, boxes, owners)
    in_maps = []
    for i in range(NCORES):
        d0 = i * SLAB
        gx = np.zeros((128, GTOT), np.float32)
        for s in range(B):
            gx[:, s * GS:(s + 1) * GS] = x[s, 0, d0:d0 + SLAB].reshape(128, GS)
        mk = np.zeros((128, MCOLS), np.float32)
        if i < len(boxes):
            gx[:, B * GS:] = bm[i]["xb"].reshape(128, BC)
            mk[:, 0 * BC:1 * BC] = bm[i]["o"].reshape(128, BC)
            mk[:, 1 * BC:2 * BC] = bm[i]["oy"].reshape(128, BC)
            mk[:, 2 * BC:3 * BC] = bm[i]["o1y"].reshape(128, BC)
            mk[:, 3 * BC:4 * BC] = bm[i]["g"].reshape(128, BC)
        in_maps.append(dict(
            gx=np.ascontiguousarray(gx.astype(BF16)),
            mk=np.ascontiguousarray(mk.astype(BF16)),
        ))
    return in_maps, bm


# --------------------------------------------------------------------------
# device kernel
# --------------------------------------------------------------------------

_BASS = {}


def _build_bass(fast=True):
    import concourse.bacc as bacc
    import concourse.tile as tile
    from concourse import mybir

    f32 = mybir.dt.float32
    bf16 = mybir.dt.bfloat16
    Alu = mybir.AluOpType
    Act = mybir.ActivationFunctionType

    HC = GS // 2  # 1024: DMA chunk cols within one sample slab

    nc = bacc.Bacc("TRN2", target_bir_lowering=False)
    gx = nc.dram_tensor("gx", [128, GTOT], bf16, kind="ExternalInput")
    mk = nc.dram_tensor("mk", [128, MCOLS], bf16, kind="ExternalInput")
    ot = nc.dram_tensor("ot", [128, OC], f32, kind="ExternalOutput")

    with tile.TileContext(nc) as tc:
        with tc.tile_pool(name="main", bufs=1) as pool, \
             tc.tile_pool(name="ps", bufs=2, space="PSUM") as ppool:

            acc = pool.tile([128, OC], f32, tag="acc")

            # ---- input tiles ----
            xb = pool.tile([128, BC], bf16, tag="xb")
            mkt = pool.tile([128, MCOLS], bf16, tag="mkt")
            xs = [[pool.tile([128, HC], bf16, tag=f"xs{s}{h}", name=f"xs{s}{h}")
                   for h in range(2)] for s in range(B)]

            # ---- DMA order: box x, s0a, mask, s0b, s1a, s1b ----
            nc.sync.dma_start(xb[:, :], gx[:, B * GS:B * GS + BC])
            nc.sync.dma_start(xs[0][0][:, :], gx[:, 0:HC])
            nc.sync.dma_start(mkt[:, :], mk[:, :])
            nc.sync.dma_start(xs[0][1][:, :], gx[:, HC:GS])
            nc.sync.dma_start(xs[1][0][:, :], gx[:, GS:GS + HC])
            nc.sync.dma_start(xs[1][1][:, :], gx[:, GS + HC:2 * GS])

            ones_bf = pool.tile([128, 1], bf16, tag="ones_bf")
            nc.gpsimd.memset(ones_bf[:, :], 1.0)
            ones_f = pool.tile([128, 1], f32, tag="ones_f")
            nc.gpsimd.memset(ones_f[:, :], 1.0)

            # one shared PSUM tile; column c -> acc column PS0 + c
            psa = ppool.tile([128, OC - PS0], f32, tag="psa")

            def colsum(src, col, nch, onet):
                """PE colsum of a [128, nch*128] fp32/bf16 region into
                psa[:, col] (chained matmuls against a ones vector)."""
                for j in range(nch):
                    nc.tensor.matmul(psa[:, col:col + 1],
                                     src[:, j * 128:(j + 1) * 128],
                                     onet[:, :], start=(j == 0),
                                     stop=(j == nch - 1))

            # ---- box ACT: u_b = exp(-x_b), l_b = ln(1+u_b) ----
            ub = pool.tile([128, BC], f32, tag="ub")
            nc.scalar.activation(ub[:, :], xb[:, :], Act.Exp, scale=-1.0)
            lb = pool.tile([128, BC], f32, tag="lb")
            nc.scalar.activation(lb[:, :], ub[:, :], Act.Ln, bias=1.0)

            # ---- box DVE: w_b = 1+u_b, p_b = 1/w_b = sigmoid(x_b) ----
            wb = pool.tile([128, BC], f32, tag="wb")
            nc.vector.tensor_scalar(wb[:, :], ub[:, :], 1.0, None, Alu.add)
            pb = pool.tile([128, BC], f32, tag="pb")
            nc.vector.reciprocal(pb[:, :], wb[:, :])

            # ---- box masked sums ----
            MO, MOY, MO1Y, MG = (mkt[:, 0:BC], mkt[:, BC:2 * BC],
                                 mkt[:, 2 * BC:3 * BC], mkt[:, 3 * BC:4 * BC])

            def msum_dve(field, mask, col, si):
                # DVE STT multiply with direct accum (valid for mult+mult)
                scr = pool.tile([128, BC], f32, tag=f"scrd{si}",
                                name=f"scrd{si}")
                nc.vector.scalar_tensor_tensor(scr[:, :], field, 1.0, mask,
                                               Alu.mult, Alu.mult,
                                               accum_out=acc[:, col:col + 1])

            def msum_pool(field, mask, pcol, si):
                # Pool tensor_tensor multiply + PE colsum into psum
                scr = pool.tile([128, BC], f32, tag=f"scrp{si}",
                                name=f"scrp{si}")
                nc.gpsimd.tensor_tensor(scr[:, :], field, mask, Alu.mult)
                colsum(scr, pcol - PS0, BC // 128, ones_f)

            msum_dve(pb[:, :], MO, C_PO, 0)
            msum_dve(pb[:, :], MOY, C_POY, 1)
            msum_dve(pb[:, :], MG, C_PG, 2)
            msum_pool(lb[:, :], MO, C_LO, 0)
            msum_pool(lb[:, :], MG, C_LG, 1)
            msum_pool(xb[:, :], MO1Y, C_X1Y, 2)
            msum_pool(xb[:, :], MG, C_XG, 3)
            msum_pool(xb[:, :], MOY, C_XOY, 4)

            # ---- global per-sample passes ----
            lsc = pool.tile([128, GS], f32, tag="lsc")   # ln scratch, reused
            us = [pool.tile([128, GS], f32, tag=f"us{s}", name=f"us{s}")
                  for s in range(B)]
            ws = [pool.tile([128, GS], f32, tag=f"ws{s}", name=f"ws{s}")
                  for s in range(B)]
            ps_t = [pool.tile([128, GS], f32, tag=f"ps_t{s}", name=f"ps_t{s}")
                    for s in range(B)]

            for s in range(B):
                # ACT exp per DMA chunk (feeds both ln and DVE early)
                for h in range(2):
                    nc.scalar.activation(us[s][:, h * HC:(h + 1) * HC],
                                         xs[s][h][:, :], Act.Exp, scale=-1.0)
                # sum x via PE colsums (bf16 weights, ones moving)
                for h in range(2):
                    for j in range(HC // 128):
                        k = h * (HC // 128) + j
                        nc.tensor.matmul(psa[:, C_X0 + s - PS0:C_X0 + s - PS0 + 1],
                                         xs[s][h][:, j * 128:(j + 1) * 128],
                                         ones_bf[:, :], start=(k == 0),
                                         stop=(k == GS // 128 - 1))
                # ACT ln with per-sample accum
                nc.scalar.activation(lsc[:, :], us[s][:, :], Act.Ln, bias=1.0,
                                     accum_out=acc[:, C_L0 + s:C_L0 + s + 1])
                # DVE: w = 1+u (one instr), p = 1/w per half (pipelines
                # behind ACT; divide/pow are invalid TSP ops, reciprocal
                # is the only DVE division) -> PE colsums give sum p
                nc.vector.tensor_scalar(ws[s][:, :], us[s][:, :], 1.0, None,
                                        Alu.add)
                for h in range(2):
                    nc.vector.reciprocal(ps_t[s][:, h * HC:(h + 1) * HC],
                                         ws[s][:, h * HC:(h + 1) * HC])
                colsum(ps_t[s], C_P0 + s - PS0, GS // 128, ones_f)

            # single psum -> acc copy (DVE; GPSIMD cannot access PSUM)
            nc.vector.tensor_scalar(acc[:, PS0:OC], psa[:, :], 1.0, None,
                                    Alu.mult)

            nc.sync.dma_start(ot[:, :], acc[:, :])

    # all our activations (Exp/Ln) live in one table; hide the other tables
    # from the act-table-load pass so it emits a single load (keeps
    # act_func_set_id indices aligned with act_info.json by preserving order)
    import concourse.bacc as _bacc_mod
    _orig_tables = _bacc_mod.get_activation_tables
    _KEEP = "natural_log_exp_and_others"

    def _only_lnexp(arch):
        tabs = _orig_tables(arch)
        assert _KEEP in tabs
        return {name: (funcs if name == _KEEP else set())
                for name, funcs in tabs.items()}

    _bacc_mod.get_activation_tables = _only_lnexp
    try:
        nc.compile()
    finally:
        _bacc_mod.get_activation_tables = _orig_tables
    return nc


def _device_partials_np(in_maps):
    """Numpy mirror of the bass kernel, for pipeline validation."""
    outs = []
    for m in in_maps:
        gxv = np.asarray(m["gx"]).astype(np.float32)
        mkv = np.asarray(m["mk"]).astype(np.float32)
        acc = np.zeros((128, OC), np.float32)
        for s in range(B):
            xsv = gxv[:, s * GS:(s + 1) * GS]
            u = np.exp(-xsv).astype(np.float32)
            l = np.log1p(u).astype(np.float32)
            p = (1.0 / (u + 1.0)).astype(np.float32)
            acc[:, C_L0 + s] = l.sum(axis=1)
            acc[:, C_X0 + s] = xsv.sum(axis=1, dtype=np.float32)
            acc[:, C_P0 + s] = p.sum(axis=1)
        xbv = gxv[:, B * GS:]
        ub = np.exp(-xbv).astype(np.float32)
        lbv = np.log1p(ub).astype(np.float32)
        pbv = (1.0 / (ub + 1.0)).astype(np.float32)
        o, oy, o1y, g = (mkv[:, 0:BC], mkv[:, BC:2 * BC],
                         mkv[:, 2 * BC:3 * BC], mkv[:, 3 * BC:4 * BC])
        acc[:, C_PO] = (pbv * o).sum(axis=1)
        acc[:, C_POY] = (pbv * oy).sum(axis=1)
        acc[:, C_PG] = (pbv * g).sum(axis=1)
        acc[:, C_LO] = (lbv * o).sum(axis=1)
        acc[:, C_LG] = (lbv * g).sum(axis=1)
        acc[:, C_X1Y] = (xbv * o1y).sum(axis=1)
        acc[:, C_XG] = (xbv * g).sum(axis=1)
        acc[:, C_XOY] = (xbv * oy).sum(axis=1)
        outs.append(dict(ot=acc))
    return outs


_PJRT = {}


def _run_pjrt_cached(nc, in_maps):
    """run_bass_via_pjrt with the jitted executable cached across calls."""
    import jax
    from jax.experimental.shard_map import shard_map
    from jax.sharding import Mesh, PartitionSpec
    from concourse import bass2jax, mybir

    key = id(nc)
    if key not in _PJRT:
        bass2jax.install_neuronx_cc_hook()
        partition_name = (nc.partition_id_tensor.name
                          if nc.partition_id_tensor else None)
        in_names, out_names, out_avals, zero_shapes = [], [], [], []
        for alloc in nc.m.functions[0].allocations:
            if not isinstance(alloc, mybir.MemoryLocationSet):
                continue
            name = alloc.memorylocations[0].name
            if alloc.kind == "ExternalInput":
                if name != partition_name:
                    in_names.append(name)
            elif alloc.kind == "ExternalOutput":
                shape = tuple(alloc.tensor_shape)
                dtype = mybir.dt.np(alloc.dtype)
                out_names.append(name)
                out_avals.append(jax.core.ShapedArray(shape, dtype))
                zero_shapes.append((shape, dtype))
        n_params = len(in_names)
        n_outs = len(out_avals)
        all_in_names = list(in_names) + list(out_names)
        if partition_name is not None:
            all_in_names.append(partition_name)

        def _body(*args):
            operands = list(args)
            if partition_name is not None:
                operands.append(bass2jax.partition_id_tensor())
            outs = bass2jax._bass_exec_p.bind(
                *operands,
                out_avals=tuple(out_avals),
                in_names=tuple(all_in_names),
                out_names=tuple(out_names),
                lowering_input_output_aliases=(),
                sim_require_finite=True,
                sim_require_nnan=True,
                nc=nc,
            )
            return tuple(outs)

        devices = jax.devices()[:NCORES]
        assert len(devices) == NCORES
        mesh = Mesh(np.asarray(devices), ("core",))
        donate = tuple(range(n_params, n_params + n_outs))
        sharded = jax.jit(
            shard_map(_body, mesh=mesh,
                      in_specs=(PartitionSpec("core"),) * (n_params + n_outs),
                      out_specs=(PartitionSpec("core"),) * n_outs,
                      check_rep=False),
            donate_argnums=donate, keep_unused=True)
        _PJRT[key] = (sharded, in_names, out_names, out_avals, zero_shapes)

    sharded, in_names, out_names, out_avals, zero_shapes = _PJRT[key]
    concat_in = [
        np.concatenate([np.asarray(m[name]) for m in in_maps], axis=0)
        for name in in_names
    ]
    concat_zeros = [
        np.zeros((NCORES * s[0], *s[1:]), dt) for s, dt in zero_shapes
    ]
    out_arrs = sharded(*concat_in, *concat_zeros)
    return [
        {name: np.asarray(out_arrs[i]).reshape(NCORES, *out_avals[i].shape)[c]
         for i, name in enumerate(out_names)}
        for c in range(NCORES)
    ]


def _device_partials(in_maps, fast=True):
    if os.environ.get("BLOB_KERNEL_NP"):
        return _device_partials_np(in_maps)
    try:
        if True not in _BASS:
            _BASS[True] = _build_bass(True)
            _BASS[False] = _BASS[True]
        return _run_pjrt_cached(_BASS[True], in_maps)
    except Exception:
        if os.environ.get("BLOB_NO_FALLBACK"):
            raise
        import traceback
        traceback.print_exc()
        print("blob kernel: device path failed; using numpy fallback",
              flush=True)
        return _device_partials_np(in_maps)


# --------------------------------------------------------------------------
# general fallback: full-volume numpy evaluation of the reference loss
# --------------------------------------------------------------------------

def _loss_general_np(x, y, meta):
    x = x.astype(np.float64)
    y = y.astype(np.float64)

    def dc_bce(xv, yv):
        # mean BCEWithLogits + batch dice over the given [b,1,D,H,W] arrays
        bce = np.mean(np.logaddexp(0.0, xv) - xv * yv)
        p = 1.0 / (1.0 + np.exp(-xv))
        I = (p * yv).sum()
        P = p.sum()
        G = yv.sum()
        dc = (2.0 * I + SMOOTH) / max(P + G + SMOOTH, 1e-8)
        return bce - dc

    global_loss = dc_bce(x, y)
    total_contrib, total_count = 0.0, 0.0
    for s in range(B):
        t8, m8, n_cc = meta[s]["t8"], meta[s]["m8"], meta[s]["n_cc"]
        xi, yi = x[s:s + 1], y[s:s + 1]
        if n_cc > 1:
            for c in range(1, n_cc + 1):
                keep = (((t8 == 0) | (t8 == c)) & ((m8 == 0) | (m8 == c))
                        ).astype(np.float64)[None, None]
                total_contrib += dc_bce(xi * keep, yi * keep)
            total_count += n_cc
        else:
            total_contrib += dc_bce(xi, yi)
            total_count += 1
    blob = total_contrib / max(total_count, 1.0)
    return 0.3 * global_loss + 0.7 * blob


# --------------------------------------------------------------------------
# public entry
# --------------------------------------------------------------------------

def kernel(net_output, target):
    x = np.ascontiguousarray(np.asarray(net_output, dtype=np.float32))
    y = np.ascontiguousarray(np.asarray(target, dtype=np.float32))
    assert x.shape == (B, 1, D, H, W) and y.shape == x.shape

    meta = _host_metadata(x, y)
    boxes, owners = _build_boxes(meta)
    general = (
        boxes is None
        or any(not m.get("crop_ok", True) for m in meta)
        or any(m["n_cc"] > K_DEV for m in meta)
        or os.environ.get("BLOB_FORCE_GENERAL")
    )
    if not general:
        ranks = _box_ranks(meta, boxes, owners)
        general = any(len(r) > 1 for r in ranks)
    if general:
        return np.asarray(_loss_general_np(x, y, meta), dtype=np.float32)

    in_maps, bm = _build_in_maps(x, y, meta, boxes, owners)
    results = _device_partials(in_maps, True)

    # ------------------------ host assembly (O(1)) ------------------------
    cols = np.zeros((NCORES, OC), np.float64)
    for i, r in enumerate(results):
        cols[i] = np.asarray(r["ot"], np.float64).sum(axis=0)

    glob = []
    for s in range(B):
        A = cols[:, C_L0 + s].sum() + cols[:, C_X0 + s].sum()
        P = cols[:, C_P0 + s].sum()
        glob.append(dict(A=A, P=P))

    XY = np.zeros(B); I = np.zeros(B); G = np.zeros(B)
    names = ["f1", "p", "py", "y", "cnt"]
    corr = [[dict(f1=0.0, p=0.0, py=0.0, y=0.0, cnt=0.0)
             for _ in range(K_DEV + 1)] for _ in range(B)]
    for i, (bs, bd, bh, bw) in enumerate(boxes):
        c = cols[i]
        cnt_o, cnt_oy, cnt_g = bm[i]["cnt_o"], bm[i]["cnt_oy"], bm[i]["cnt_g"]
        own = dict(f1=c[C_LO] + c[C_X1Y], p=c[C_PO],
                   py=c[C_POY], y=cnt_oy, cnt=cnt_o)
        bg = dict(f1=c[C_LG] + c[C_XG], p=c[C_PG],
                  py=0.0, y=0.0, cnt=cnt_g)
        XY[bs] += c[C_XOY]
        I[bs] += own["py"]
        G[bs] += cnt_oy
        rset = ranks[i]
        for lab in range(1, K_DEV + 1):
            kp = own if (rset and lab in rset) else bg
            for nm in names:
                corr[bs][lab][nm] += kp[nm] - own[nm]

    total_contrib = 0.0
    total_count = 0.0
    for s in range(B):
        n_cc = meta[s]["n_cc"]
        gg = dict(f1=glob[s]["A"] - XY[s], p=glob[s]["P"], py=I[s], y=G[s],
                  cnt=float(N))
        if n_cc > 1:
            for lab in range(1, n_cc + 1):
                Sf = {nm: gg[nm] + corr[s][lab][nm] for nm in names}
                nk = Sf["cnt"]
                bce = (Sf["f1"] + LOG2 * (N - nk)) / N
                Pc = Sf["p"] + 0.5 * (N - nk)
                dc = (2.0 * Sf["py"] + SMOOTH) / max(Pc + Sf["y"] + SMOOTH, 1e-8)
                total_contrib += bce - dc
            total_count += n_cc
        else:
            bce = gg["f1"] / N
            dc = (2.0 * gg["py"] + SMOOTH) / max(gg["p"] + gg["y"] + SMOOTH, 1e-8)
            total_contrib += bce - dc
            total_count += 1

    f1b = sum(glob[s]["A"] - XY[s] for s in range(B))
    bce_g = f1b / (B * N)
    Ib = I.sum(); Pb = sum(g["P"] for g in glob); Gb = G.sum()
    dc_g = (2.0 * Ib + SMOOTH) / max(Pb + Gb + SMOOTH, 1e-8)
    global_loss = bce_g - dc_g

    blob = total_contrib / max(total_count, 1.0)
    out = 0.3 * global_loss + 0.7 * blob
    return np.asarray(out, dtype=np.float32)


# revision 11
# speedup vs baseline: 1.0410x; 1.0410x over previous
"""Bass/Trainium2 kernel for nn_Blob_DC_and_BCE_loss (loss_fn).

Strategy (v2)
-------------
The loss decomposes into per-sample global sums plus small ROI-box
corrections.  Only three global per-voxel quantities are needed:

    A_s = sum softplus(x)   (for BCE)
    P_s = sum sigmoid(x)    (for dice denominators)
    Sx_s = sum x

because every y=1 voxel lives inside a target component, which is fully
covered by the ROI boxes: sum x*y, sum p*y and sum y are recovered from
per-box masked sums plus host-side integer counts.

Identities used on device (one Exp+Ln activation table, no reloads):
    u = exp(-x)            ACT
    l = ln(1+u)            ACT  (= softplus(-x); accum -> sum per sample)
    p = reciprocal(1+u)    DVE  (= sigmoid(x); PE colsums -> sum per sample)
    A_s = sum l + sum x    (sum x via PE matmul colsums, free elsewhere)
    P_s = sum p

ROI boxes (one 32^3 box per core): host builds 0/1 masks
    o  = owned, oy = owned&y, o1y = owned&(1-y), g = owned&t==0&m==0
and the device computes eight masked sums with single-instruction
scalar_tensor_tensor multiply+accumulate, split across DVE and Pool.

All DMA-touched tensors are bf16 (x, masks); all on-chip intermediates
are fp32 (chained bf16 rounding of ln(1+u) biases A by ~1%, far above
budget).  I/O is batched: 6 input DMA chunks, one [128,14] fp32 output
DMA per core (the v1 kernel's 42 tiny DMAs serialized ~27us on the SP
sequencer + HWDGE).

Host: CC labeling, box/ownership setup, integer mask counts, O(1)
scalar assembly.  Device: all O(N) float math.
"""

import math
import os

import numpy as np

try:
    import ml_dtypes

    BF16 = ml_dtypes.bfloat16
except Exception:  # pragma: no cover
    BF16 = None

B = 2
D = H = W = 128
N = D * H * W
NCORES = 8
SLAB = D // NCORES            # 16 depth slices per core
GS = SLAB * H * W // 128      # 2048: free-dim cols of one sample slab
BOX = 32                      # ROI box edge
BC = BOX ** 3 // 128          # 256: free-dim cols of one box
GTOT = B * GS + BC            # 4352 cols in the x input tensor
MCOLS = 4 * BC                # 1024 cols in the mask tensor (o|oy|o1y|g)
K_DEV = 4                     # labels per sample handled in fast path
LOG2 = math.log(2.0)
SMOOTH = 1e-5
NSLAB = 128 * GS              # voxels per core per sample

# output columns of the per-core [128, OC] accumulator.
# cols 0-4: direct accum_out writes (ACT ln accums + DVE STT msum accums);
# cols 5-13: PE-colsum results copied from one [128,9] PSUM tile.
C_PO, C_POY, C_PG = 0, 1, 2   # box sigmoid(x) masked sums (DVE STT accum)
C_X0, C_X1 = 3, 4             # sum x per sample (PE colsum)
C_P0, C_P1 = 5, 6             # sum sigmoid(x) per sample (PE colsum)
C_L0, C_L1 = 7, 8             # sum ln(1+u) per sample (PE colsum)
C_LO, C_LG = 9, 10            # box ln masked sums (Pool TT + PE colsum)
C_X1Y, C_XG, C_XOY = 11, 12, 13  # box x masked sums (Pool TT + PE colsum)
OC = 14
PS0 = C_X0                    # psum column c maps to acc column PS0 + c


# --------------------------------------------------------------------------
# host-side connected components (scipy if present, numpy fallback)
# --------------------------------------------------------------------------

def _label_np(mask):
    """6-connectivity CC labeling, pure numpy (iterative min-propagation)."""
    lab = np.where(mask, np.arange(1, mask.size + 1, dtype=np.int64
                                   ).reshape(mask.shape), 0)
    while True:
        new = lab.copy()
        sl = new[1:, :, :]; np.minimum(sl, np.where(lab[:-1] > 0, lab[:-1], sl), out=sl)
        sl = new[:-1, :, :]; np.minimum(sl, np.where(lab[1:] > 0, lab[1:], sl), out=sl)
        sl = new[:, 1:, :]; np.minimum(sl, np.where(lab[:, :-1] > 0, lab[:, :-1], sl), out=sl)
        sl = new[:, :-1, :]; np.minimum(sl, np.where(lab[:, 1:] > 0, lab[:, 1:], sl), out=sl)
        sl = new[:, :, 1:]; np.minimum(sl, np.where(lab[:, :, :-1] > 0, lab[:, :, :-1], sl), out=sl)
        sl = new[:, :, :-1]; np.minimum(sl, np.where(lab[:, :, 1:] > 0, lab[:, :, 1:], sl), out=sl)
        new = np.where(mask, new, 0)
        if np.array_equal(new, lab):
            break
        lab = new
    uniq = np.unique(lab[lab > 0])
    remap = np.zeros(int(lab.max()) + 1, np.int64)
    remap[uniq] = np.arange(1, len(uniq) + 1)
    return remap[lab], len(uniq)


def _cc_label(mask):
    try:
        from scipy import ndimage as ndi
        st = ndi.generate_binary_structure(3, 1)
        lab, n = ndi.label(mask, structure=st)
        return lab.astype(np.int64), int(n)
    except Exception:
        return _label_np(mask)


CROP_MARGIN = 24   # predicted comps matched to a target stay well inside this


def _host_metadata(x, y):
    """Per-sample rank volumes t8/m8 and component counts.

    All labeling runs on a crop = target bounding box + CROP_MARGIN.  A
    predicted component can only be matched to a target if it intersects
    it, and matched components are small appendages of the targets, so
    anything outside the crop has t = m = 0.  The crop assumption is
    verified (no predicted foreground on the crop faces is labeled).
    """
    meta = []
    for b in range(B):
        tgt_full = y[b, 0] > 0.5
        pred_full = x[b, 0] >= 0.0
        if not tgt_full.any():
            meta.append(dict(t8=np.zeros((D, H, W), np.float32),
                             m8=np.zeros((D, H, W), np.float32), n_cc=0))
            continue
        idx = np.argwhere(tgt_full)
        lo = np.maximum(idx.min(axis=0) - CROP_MARGIN, 0)
        hi = np.minimum(idx.max(axis=0) + 1 + CROP_MARGIN, (D, H, W))
        sl = tuple(slice(int(a), int(c)) for a, c in zip(lo, hi))
        tgt = tgt_full[sl]
        pred = pred_full[sl]
        lin1 = (np.arange(N, dtype=np.int64).reshape(D, H, W)[sl] + 1)
        tlab, ntc = _cc_label(tgt)
        plab, npc = _cc_label(pred)
        # reference label value = max linear index + 1 within target comp
        tmax = np.zeros(ntc + 1, np.int64)
        np.maximum.at(tmax, tlab.ravel(), np.where(tgt, lin1, 0).ravel())
        tval = np.where(tgt, tmax[tlab], 0)
        # map each predicted comp to the max target label it overlaps
        pmax = np.zeros(npc + 1, np.int64)
        np.maximum.at(pmax, plab.ravel(), tval.ravel())
        mval = np.where(pred, pmax[plab], 0)
        # crop-validity: no matched predicted voxel may touch a crop face
        ok = True
        for ax in range(3):
            for face in (0, -1):
                f = [slice(None)] * 3
                f[ax] = face
                if (mval[tuple(f)] > 0).any():
                    ok = False
        # ranks: descending reference label order (top_k order)
        labels_desc = np.sort(np.unique(tval[tval > 0]))[::-1]
        n_cc = len(labels_desc)
        rank_of = np.zeros(int(tval.max()) + 1 if n_cc else 1, np.int64)
        for i, L in enumerate(labels_desc):
            rank_of[L] = i + 1
        t8 = np.zeros((D, H, W), np.float32)
        m8 = np.zeros((D, H, W), np.float32)
        t8[sl] = rank_of[tval]
        m8[sl] = rank_of[mval]
        meta.append(dict(t8=t8, m8=m8, n_cc=n_cc, crop_ok=ok))
    return meta


def _build_boxes(meta):
    """Cover the interesting voxels with <= NCORES boxes of BOX^3.

    Each connected cluster of the interesting set (target comp + its
    matched predicted comps) is covered by a grid of boxes over its bbox.
    Returns list of (sample, d0, h0, w0) and per-sample ownership arrays
    (box index owning each voxel, -1 if none).  Returns (None, None) when
    more than NCORES boxes would be needed (general fallback).
    """
    boxes = []
    owners = []
    for b in range(B):
        t8, m8 = meta[b]["t8"], meta[b]["m8"]
        interesting = (t8 > 0) | (m8 > 0)
        own = np.full((D, H, W), -1, np.int32)
        owners.append(own)
        if not interesting.any():
            continue
        clab, ncl = _cc_label(interesting)
        sample_boxes = []
        for ci in range(1, ncl + 1):
            idx = np.argwhere(clab == ci)
            lo, hi = idx.min(axis=0), idx.max(axis=0)  # inclusive
            starts_per_dim = []
            for ax in range(3):
                ext = int(hi[ax] - lo[ax] + 1)
                nb = (ext + BOX - 1) // BOX
                if nb == 1:
                    s0 = int(lo[ax]) - (BOX - ext) // 2
                    starts_per_dim.append([min(max(s0, 0), D - BOX)])
                else:
                    step = (ext - BOX) / (nb - 1)
                    starts_per_dim.append(
                        [min(max(int(lo[ax] + round(i * step)), 0), D - BOX)
                         for i in range(nb)])
            for sd in starts_per_dim[0]:
                for sh in starts_per_dim[1]:
                    for sw in starts_per_dim[2]:
                        bi = len(boxes)
                        if bi >= NCORES:
                            return None, None
                        boxes.append((b, sd, sh, sw))
                        sample_boxes.append((bi, ci, sd, sh, sw))
                        # interesting voxels of THIS cluster claim the box
                        sl = (slice(sd, sd + BOX), slice(sh, sh + BOX),
                              slice(sw, sw + BOX))
                        region = own[sl]
                        region[(clab[sl] == ci) & (region < 0)] = bi
        # background (non-interesting) voxels: first covering box wins
        for bi, ci, sd, sh, sw in sample_boxes:
            sl = (slice(sd, sd + BOX), slice(sh, sh + BOX),
                  slice(sw, sw + BOX))
            region = own[sl]
            region[region < 0] = bi
    for b in range(B):
        t8, m8 = meta[b]["t8"], meta[b]["m8"]
        if (((t8 > 0) | (m8 > 0)) & (owners[b] < 0)).any():
            return None, None
    return boxes, owners


def _box_ranks(meta, boxes, owners):
    """Per box: set of component ranks present among its owned voxels."""
    ranks = []
    for i, (bsmp, bd, bh, bw) in enumerate(boxes):
        sl = (slice(bd, bd + BOX), slice(bh, bh + BOX), slice(bw, bw + BOX))
        owned = owners[bsmp][sl] == i
        t = meta[bsmp]["t8"][sl][owned]
        m = meta[bsmp]["m8"][sl][owned]
        rs = set(np.unique(t[t > 0]).tolist()) | set(np.unique(m[m > 0]).tolist())
        ranks.append({int(r) for r in rs})
    return ranks


def _box_masks_counts(x, y, meta, boxes, owners):
    """Per box: the four bf16 0/1 mask planes + integer counts."""
    out = []
    for i, (bs, bd, bh, bw) in enumerate(boxes):
        sl = (slice(bd, bd + BOX), slice(bh, bh + BOX), slice(bw, bw + BOX))
        owned = owners[bs][sl] == i
        t8 = meta[bs]["t8"][sl]
        m8 = meta[bs]["m8"][sl]
        yb = y[bs, 0][sl] > 0.5
        o = owned
        oy = owned & yb
        o1y = owned & ~yb
        g = owned & (t8 == 0) & (m8 == 0)
        out.append(dict(
            o=o, oy=oy, o1y=o1y, g=g,
            cnt_o=float(o.sum()), cnt_oy=float(oy.sum()),
            cnt_g=float(g.sum()),
            xb=x[bs, 0][sl],
        ))
    return out


def _build_in_maps(x, y, meta, boxes, owners):
    """Per-core input tensors: gx [128, GTOT] bf16, mk [128, MCOLS] bf16."""
    bm = _box_masks_counts(x, y, meta, boxes, owners)
    in_maps = []
    for i in range(NCORES):
        d0 = i * SLAB
        gx = np.zeros((128, GTOT), np.float32)
        for s in range(B):
            gx[:, s * GS:(s + 1) * GS] = x[s, 0, d0:d0 + SLAB].reshape(128, GS)
        mk = np.zeros((128, MCOLS), np.float32)
        if i < len(boxes):
            gx[:, B * GS:] = bm[i]["xb"].reshape(128, BC)
            mk[:, 0 * BC:1 * BC] = bm[i]["o"].reshape(128, BC)
            mk[:, 1 * BC:2 * BC] = bm[i]["oy"].reshape(128, BC)
            mk[:, 2 * BC:3 * BC] = bm[i]["o1y"].reshape(128, BC)
            mk[:, 3 * BC:4 * BC] = bm[i]["g"].reshape(128, BC)
        in_maps.append(dict(
            gx=np.ascontiguousarray(gx.astype(BF16)),
            mk=np.ascontiguousarray(mk.astype(BF16)),
        ))
    return in_maps, bm


# --------------------------------------------------------------------------
# device kernel
# --------------------------------------------------------------------------

_BASS = {}


def _build_bass(fast=True):
    import concourse.bacc as bacc
    import concourse.tile as tile
    from concourse import mybir

    f32 = mybir.dt.float32
    bf16 = mybir.dt.bfloat16
    Alu = mybir.AluOpType
    Act = mybir.ActivationFunctionType

    HC = GS // 2  # 1024: DMA chunk cols within one sample slab

    nc = bacc.Bacc("TRN2", target_bir_lowering=False)
    gx = nc.dram_tensor("gx", [128, GTOT], bf16, kind="ExternalInput")
    mk = nc.dram_tensor("mk", [128, MCOLS], bf16, kind="ExternalInput")
    ot = nc.dram_tensor("ot", [128, OC], f32, kind="ExternalOutput")

    with tile.TileContext(nc) as tc:
        with tc.tile_pool(name="main", bufs=1) as pool, \
             tc.tile_pool(name="ps", bufs=2, space="PSUM") as ppool:

            acc = pool.tile([128, OC], f32, tag="acc")

            # ---- input tiles ----
            xb = pool.tile([128, BC], bf16, tag="xb")
            mkt = pool.tile([128, MCOLS], bf16, tag="mkt")
            xs = [[pool.tile([128, HC], bf16, tag=f"xs{s}{h}", name=f"xs{s}{h}")
                   for h in range(2)] for s in range(B)]

            # ---- DMA order: s0a first (earliest ACT start), box + mask
            # early enough for the box side work, then s0b, s1a, s1b ----
            nc.sync.dma_start(xs[0][0][:, :], gx[:, 0:HC])
            nc.sync.dma_start(xb[:, :], gx[:, B * GS:B * GS + BC])
            nc.sync.dma_start(xs[0][1][:, :], gx[:, HC:GS])
            nc.sync.dma_start(mkt[:, :], mk[:, :])
            nc.sync.dma_start(xs[1][0][:, :], gx[:, GS:GS + HC])
            nc.sync.dma_start(xs[1][1][:, :], gx[:, GS + HC:2 * GS])

            ones_bf = pool.tile([128, 1], bf16, tag="ones_bf")
            nc.gpsimd.memset(ones_bf[:, :], 1.0)
            ones_f = pool.tile([128, 1], f32, tag="ones_f")
            nc.gpsimd.memset(ones_f[:, :], 1.0)

            # one shared PSUM tile; column c -> acc column PS0 + c
            psa = ppool.tile([128, OC - PS0], f32, tag="psa")

            def colsum(src, col, nch, onet):
                """PE colsum of a [128, nch*128] fp32/bf16 region into
                psa[:, col] (chained matmuls against a ones vector)."""
                for j in range(nch):
                    nc.tensor.matmul(psa[:, col:col + 1],
                                     src[:, j * 128:(j + 1) * 128],
                                     onet[:, :], start=(j == 0),
                                     stop=(j == nch - 1))

            us = [pool.tile([128, GS], f32, tag=f"us{s}", name=f"us{s}")
                  for s in range(B)]
            ls = [pool.tile([128, GS], f32, tag=f"ls{s}", name=f"ls{s}")
                  for s in range(B)]
            ws = [pool.tile([128, GS], f32, tag=f"ws{s}", name=f"ws{s}")
                  for s in range(B)]
            ps_t = [pool.tile([128, GS], f32, tag=f"ps_t{s}", name=f"ps_t{s}")
                    for s in range(B)]
            ub = pool.tile([128, BC], f32, tag="ub")
            lb = pool.tile([128, BC], f32, tag="lb")
            wb = pool.tile([128, BC], f32, tag="wb")
            pb = pool.tile([128, BC], f32, tag="pb")

            # ---- ACT stream: all exps before the two ln passes, so the
            # DVE reciprocal chains (which only need u) start early and the
            # final P colsums are not pushed past the last ln ----
            nc.scalar.activation(us[0][:, 0:HC], xs[0][0][:, :], Act.Exp,
                                 scale=-1.0)
            nc.scalar.activation(ub[:, :], xb[:, :], Act.Exp, scale=-1.0)
            nc.scalar.activation(lb[:, :], ub[:, :], Act.Ln, bias=1.0)
            nc.scalar.activation(us[0][:, HC:GS], xs[0][1][:, :], Act.Exp,
                                 scale=-1.0)
            nc.scalar.activation(us[1][:, 0:HC], xs[1][0][:, :], Act.Exp,
                                 scale=-1.0)
            nc.scalar.activation(us[1][:, HC:GS], xs[1][1][:, :], Act.Exp,
                                 scale=-1.0)
            nc.scalar.activation(ls[0][:, :], us[0][:, :], Act.Ln, bias=1.0)
            nc.scalar.activation(ls[1][:, :], us[1][:, :], Act.Ln, bias=1.0)

            # ---- DVE stream: box chain, box p-msums, then the two
            # w+reciprocal chains (per-half recips pipeline behind ACT) ----
            MO, MOY, MO1Y, MG = (mkt[:, 0:BC], mkt[:, BC:2 * BC],
                                 mkt[:, 2 * BC:3 * BC], mkt[:, 3 * BC:4 * BC])

            nc.vector.tensor_scalar(wb[:, :], ub[:, :], 1.0, None, Alu.add)
            nc.vector.reciprocal(pb[:, :], wb[:, :])

            def msum_dve(field, mask, col, si):
                scr = pool.tile([128, BC], f32, tag=f"scrd{si}",
                                name=f"scrd{si}")
                nc.vector.scalar_tensor_tensor(scr[:, :], field, 1.0, mask,
                                               Alu.mult, Alu.mult,
                                               accum_out=acc[:, col:col + 1])

            msum_dve(pb[:, :], MO, C_PO, 0)
            msum_dve(pb[:, :], MOY, C_POY, 1)
            msum_dve(pb[:, :], MG, C_PG, 2)

            for s in range(B):
                nc.vector.tensor_scalar(ws[s][:, :], us[s][:, :], 1.0, None,
                                        Alu.add)
                for h in range(2):
                    nc.vector.reciprocal(ps_t[s][:, h * HC:(h + 1) * HC],
                                         ws[s][:, h * HC:(h + 1) * HC])

            # ---- Pool stream: box l/x masked products (PE colsums them) ----
            pscr = []
            for si, (field, mask) in enumerate([(lb, MO), (lb, MG), (xb, MO1Y),
                                                (xb, MG), (xb, MOY)]):
                scr = pool.tile([128, BC], f32, tag=f"scrp{si}",
                                name=f"scrp{si}")
                nc.gpsimd.tensor_tensor(scr[:, :], field[:, :], mask, Alu.mult)
                pscr.append(scr)

            # ---- PE stream, ordered by data availability ----
            for s in range(B):
                for h in range(2):
                    for j in range(HC // 128):
                        k = h * (HC // 128) + j
                        nc.tensor.matmul(
                            psa[:, C_X0 + s - PS0:C_X0 + s - PS0 + 1],
                            xs[s][h][:, j * 128:(j + 1) * 128],
                            ones_bf[:, :], start=(k == 0),
                            stop=(k == GS // 128 - 1))
            for si, col in enumerate([C_LO, C_LG, C_X1Y, C_XG, C_XOY]):
                colsum(pscr[si], col - PS0, BC // 128, ones_f)
            for s in range(B):
                colsum(ps_t[s], C_P0 + s - PS0, GS // 128, ones_f)
                colsum(ls[s], C_L0 + s - PS0, GS // 128, ones_f)

            # single psum -> acc copy (DVE; GPSIMD cannot access PSUM)
            nc.vector.tensor_scalar(acc[:, PS0:OC], psa[:, :], 1.0, None,
                                    Alu.mult)

            nc.sync.dma_start(ot[:, :], acc[:, :])

    # all our activations (Exp/Ln) live in one table; hide the other tables
    # from the act-table-load pass so it emits a single load (keeps
    # act_func_set_id indices aligned with act_info.json by preserving order)
    import concourse.bacc as _bacc_mod
    _orig_tables = _bacc_mod.get_activation_tables
    _KEEP = "natural_log_exp_and_others"

    def _only_lnexp(arch):
        tabs = _orig_tables(arch)
        assert _KEEP in tabs
        return {name: (funcs if name == _KEEP else set())
                for name, funcs in tabs.items()}

    _bacc_mod.get_activation_tables = _only_lnexp
    try:
        nc.compile()
    finally:
        _bacc_mod.get_activation_tables = _orig_tables
    return nc


def _device_partials_np(in_maps):
    """Numpy mirror of the bass kernel, for pipeline validation."""
    outs = []
    for m in in_maps:
        gxv = np.asarray(m["gx"]).astype(np.float32)
        mkv = np.asarray(m["mk"]).astype(np.float32)
        acc = np.zeros((128, OC), np.float32)
        for s in range(B):
            xsv = gxv[:, s * GS:(s + 1) * GS]
            u = np.exp(-xsv).astype(np.float32)
            l = np.log1p(u).astype(np.float32)
            p = (1.0 / (u + 1.0)).astype(np.float32)
            acc[:, C_L0 + s] = l.sum(axis=1)
            acc[:, C_X0 + s] = xsv.sum(axis=1, dtype=np.float32)
            acc[:, C_P0 + s] = p.sum(axis=1)
        xbv = gxv[:, B * GS:]
        ub = np.exp(-xbv).astype(np.float32)
        lbv = np.log1p(ub).astype(np.float32)
        pbv = (1.0 / (ub + 1.0)).astype(np.float32)
        o, oy, o1y, g = (mkv[:, 0:BC], mkv[:, BC:2 * BC],
                         mkv[:, 2 * BC:3 * BC], mkv[:, 3 * BC:4 * BC])
        acc[:, C_PO] = (pbv * o).sum(axis=1)
        acc[:, C_POY] = (pbv * oy).sum(axis=1)
        acc[:, C_PG] = (pbv * g).sum(axis=1)
        acc[:, C_LO] = (lbv * o).sum(axis=1)
        acc[:, C_LG] = (lbv * g).sum(axis=1)
        acc[:, C_X1Y] = (xbv * o1y).sum(axis=1)
        acc[:, C_XG] = (xbv * g).sum(axis=1)
        acc[:, C_XOY] = (xbv * oy).sum(axis=1)
        outs.append(dict(ot=acc))
    return outs


_PJRT = {}


def _run_pjrt_cached(nc, in_maps):
    """run_bass_via_pjrt with the jitted executable cached across calls."""
    import jax
    from jax.experimental.shard_map import shard_map
    from jax.sharding import Mesh, PartitionSpec
    from concourse import bass2jax, mybir

    key = id(nc)
    if key not in _PJRT:
        bass2jax.install_neuronx_cc_hook()
        partition_name = (nc.partition_id_tensor.name
                          if nc.partition_id_tensor else None)
        in_names, out_names, out_avals, zero_shapes = [], [], [], []
        for alloc in nc.m.functions[0].allocations:
            if not isinstance(alloc, mybir.MemoryLocationSet):
                continue
            name = alloc.memorylocations[0].name
            if alloc.kind == "ExternalInput":
                if name != partition_name:
                    in_names.append(name)
            elif alloc.kind == "ExternalOutput":
                shape = tuple(alloc.tensor_shape)
                dtype = mybir.dt.np(alloc.dtype)
                out_names.append(name)
                out_avals.append(jax.core.ShapedArray(shape, dtype))
                zero_shapes.append((shape, dtype))
        n_params = len(in_names)
        n_outs = len(out_avals)
        all_in_names = list(in_names) + list(out_names)
        if partition_name is not None:
            all_in_names.append(partition_name)

        def _body(*args):
            operands = list(args)
            if partition_name is not None:
                operands.append(bass2jax.partition_id_tensor())
            outs = bass2jax._bass_exec_p.bind(
                *operands,
                out_avals=tuple(out_avals),
                in_names=tuple(all_in_names),
                out_names=tuple(out_names),
                lowering_input_output_aliases=(),
                sim_require_finite=True,
                sim_require_nnan=True,
                nc=nc,
            )
            return tuple(outs)

        devices = jax.devices()[:NCORES]
        assert len(devices) == NCORES
        mesh = Mesh(np.asarray(devices), ("core",))
        donate = tuple(range(n_params, n_params + n_outs))
        sharded = jax.jit(
            shard_map(_body, mesh=mesh,
                      in_specs=(PartitionSpec("core"),) * (n_params + n_outs),
                      out_specs=(PartitionSpec("core"),) * n_outs,
                      check_rep=False),
            donate_argnums=donate, keep_unused=True)
        _PJRT[key] = (sharded, in_names, out_names, out_avals, zero_shapes)

    sharded, in_names, out_names, out_avals, zero_shapes = _PJRT[key]
    concat_in = [
        np.concatenate([np.asarray(m[name]) for m in in_maps], axis=0)
        for name in in_names
    ]
    concat_zeros = [
        np.zeros((NCORES * s[0], *s[1:]), dt) for s, dt in zero_shapes
    ]
    out_arrs = sharded(*concat_in, *concat_zeros)
    return [
        {name: np.asarray(out_arrs[i]).reshape(NCORES, *out_avals[i].shape)[c]
         for i, name in enumerate(out_names)}
        for c in range(NCORES)
    ]


def _device_partials(in_maps, fast=True):
    if os.environ.get("BLOB_KERNEL_NP"):
        return _device_partials_np(in_maps)
    try:
        if True not in _BASS:
            _BASS[True] = _build_bass(True)
            _BASS[False] = _BASS[True]
        return _run_pjrt_cached(_BASS[True], in_maps)
    except Exception:
        if os.environ.get("BLOB_NO_FALLBACK"):
            raise
        import traceback
        traceback.print_exc()
        print("blob kernel: device path failed; using numpy fallback",
              flush=True)
        return _device_partials_np(in_maps)


# --------------------------------------------------------------------------
# general fallback: full-volume numpy evaluation of the reference loss
# --------------------------------------------------------------------------

def _loss_general_np(x, y, meta):
    x = x.astype(np.float64)
    y = y.astype(np.float64)

    def dc_bce(xv, yv):
        # mean BCEWithLogits + batch dice over the given [b,1,D,H,W] arrays
        bce = np.mean(np.logaddexp(0.0, xv) - xv * yv)
        p = 1.0 / (1.0 + np.exp(-xv))
        I = (p * yv).sum()
        P = p.sum()
        G = yv.sum()
        dc = (2.0 * I + SMOOTH) / max(P + G + SMOOTH, 1e-8)
        return bce - dc

    global_loss = dc_bce(x, y)
    total_contrib, total_count = 0.0, 0.0
    for s in range(B):
        t8, m8, n_cc = meta[s]["t8"], meta[s]["m8"], meta[s]["n_cc"]
        xi, yi = x[s:s + 1], y[s:s + 1]
        if n_cc > 1:
            for c in range(1, n_cc + 1):
                keep = (((t8 == 0) | (t8 == c)) & ((m8 == 0) | (m8 == c))
                        ).astype(np.float64)[None, None]
                total_contrib += dc_bce(xi * keep, yi * keep)
            total_count += n_cc
        else:
            total_contrib += dc_bce(xi, yi)
            total_count += 1
    blob = total_contrib / max(total_count, 1.0)
    return 0.3 * global_loss + 0.7 * blob


# --------------------------------------------------------------------------
# public entry
# --------------------------------------------------------------------------

def kernel(net_output, target):
    x = np.ascontiguousarray(np.asarray(net_output, dtype=np.float32))
    y = np.ascontiguousarray(np.asarray(target, dtype=np.float32))
    assert x.shape == (B, 1, D, H, W) and y.shape == x.shape

    meta = _host_metadata(x, y)
    boxes, owners = _build_boxes(meta)
    general = (
        boxes is None
        or any(not m.get("crop_ok", True) for m in meta)
        or any(m["n_cc"] > K_DEV for m in meta)
        or os.environ.get("BLOB_FORCE_GENERAL")
    )
    if not general:
        ranks = _box_ranks(meta, boxes, owners)
        general = any(len(r) > 1 for r in ranks)
    if general:
        return np.asarray(_loss_general_np(x, y, meta), dtype=np.float32)

    in_maps, bm = _build_in_maps(x, y, meta, boxes, owners)
    results = _device_partials(in_maps, True)

    # ------------------------ host assembly (O(1)) ------------------------
    cols = np.zeros((NCORES, OC), np.float64)
    for i, r in enumerate(results):
        cols[i] = np.asarray(r["ot"], np.float64).sum(axis=0)

    glob = []
    for s in range(B):
        A = cols[:, C_L0 + s].sum() + cols[:, C_X0 + s].sum()
        P = cols[:, C_P0 + s].sum()
        glob.append(dict(A=A, P=P))

    XY = np.zeros(B); I = np.zeros(B); G = np.zeros(B)
    names = ["f1", "p", "py", "y", "cnt"]
    corr = [[dict(f1=0.0, p=0.0, py=0.0, y=0.0, cnt=0.0)
             for _ in range(K_DEV + 1)] for _ in range(B)]
    for i, (bs, bd, bh, bw) in enumerate(boxes):
        c = cols[i]
        cnt_o, cnt_oy, cnt_g = bm[i]["cnt_o"], bm[i]["cnt_oy"], bm[i]["cnt_g"]
        own = dict(f1=c[C_LO] + c[C_X1Y], p=c[C_PO],
                   py=c[C_POY], y=cnt_oy, cnt=cnt_o)
        bg = dict(f1=c[C_LG] + c[C_XG], p=c[C_PG],
                  py=0.0, y=0.0, cnt=cnt_g)
        XY[bs] += c[C_XOY]
        I[bs] += own["py"]
        G[bs] += cnt_oy
        rset = ranks[i]
        for lab in range(1, K_DEV + 1):
            kp = own if (rset and lab in rset) else bg
            for nm in names:
                corr[bs][lab][nm] += kp[nm] - own[nm]

    total_contrib = 0.0
    total_count = 0.0
    for s in range(B):
        n_cc = meta[s]["n_cc"]
        gg = dict(f1=glob[s]["A"] - XY[s], p=glob[s]["P"], py=I[s], y=G[s],
                  cnt=float(N))
        if n_cc > 1:
            for lab in range(1, n_cc + 1):
                Sf = {nm: gg[nm] + corr[s][lab][nm] for nm in names}
                nk = Sf["cnt"]
                bce = (Sf["f1"] + LOG2 * (N - nk)) / N
                Pc = Sf["p"] + 0.5 * (N - nk)
                dc = (2.0 * Sf["py"] + SMOOTH) / max(Pc + Sf["y"] + SMOOTH, 1e-8)
                total_contrib += bce - dc
            total_count += n_cc
        else:
            bce = gg["f1"] / N
            dc = (2.0 * gg["py"] + SMOOTH) / max(gg["p"] + gg["y"] + SMOOTH, 1e-8)
            total_contrib += bce - dc
            total_count += 1

    f1b = sum(glob[s]["A"] - XY[s] for s in range(B))
    bce_g = f1b / (B * N)
    Ib = I.sum(); Pb = sum(g["P"] for g in glob); Gb = G.sum()
    dc_g = (2.0 * Ib + SMOOTH) / max(Pb + Gb + SMOOTH, 1e-8)
    global_loss = bce_g - dc_g

    blob = total_contrib / max(total_count, 1.0)
    out = 0.3 * global_loss + 0.7 * blob
    return np.asarray(out, dtype=np.float32)


# revision 13
# speedup vs baseline: 1.0994x; 1.0561x over previous
"""Bass/Trainium2 kernel for nn_Blob_DC_and_BCE_loss (loss_fn).

Strategy (v2)
-------------
The loss decomposes into per-sample global sums plus small ROI-box
corrections.  Only three global per-voxel quantities are needed:

    A_s = sum softplus(x)   (for BCE)
    P_s = sum sigmoid(x)    (for dice denominators)
    Sx_s = sum x

because every y=1 voxel lives inside a target component, which is fully
covered by the ROI boxes: sum x*y, sum p*y and sum y are recovered from
per-box masked sums plus host-side integer counts.

Identities used on device (one Exp+Ln activation table, no reloads):
    u = exp(-x)            ACT
    l = ln(1+u)            ACT  (= softplus(-x); accum -> sum per sample)
    p = reciprocal(1+u)    DVE  (= sigmoid(x); PE colsums -> sum per sample)
    A_s = sum l + sum x    (sum x via PE matmul colsums, free elsewhere)
    P_s = sum p

ROI boxes (one 32^3 box per core): host builds 0/1 masks
    o  = owned, oy = owned&y, o1y = owned&(1-y), g = owned&t==0&m==0
and the device computes eight masked sums with single-instruction
scalar_tensor_tensor multiply+accumulate, split across DVE and Pool.

All DMA-touched tensors are bf16 (x, masks); all on-chip intermediates
are fp32 (chained bf16 rounding of ln(1+u) biases A by ~1%, far above
budget).  I/O is batched: 6 input DMA chunks, one [128,14] fp32 output
DMA per core (the v1 kernel's 42 tiny DMAs serialized ~27us on the SP
sequencer + HWDGE).

Host: CC labeling, box/ownership setup, integer mask counts, O(1)
scalar assembly.  Device: all O(N) float math.
"""

import math
import os

import numpy as np

try:
    import ml_dtypes

    BF16 = ml_dtypes.bfloat16
except Exception:  # pragma: no cover
    BF16 = None

B = 2
D = H = W = 128
N = D * H * W
NCORES = 8
SLAB = D // NCORES            # 16 depth slices per core
GS = SLAB * H * W // 128      # 2048: free-dim cols of one sample slab
BOX = 32                      # ROI box edge
BC = BOX ** 3 // 128          # 256: free-dim cols of one box
GTOT = B * GS + BC            # 4352 cols in the x input tensor
MCOLS = 4 * BC                # 1024 cols in the mask tensor (o|oy|o1y|g)
K_DEV = 4                     # labels per sample handled in fast path
LOG2 = math.log(2.0)
SMOOTH = 1e-5
NSLAB = 128 * GS              # voxels per core per sample

# output columns of the per-core [128, OC] accumulator.
# cols 0-4: direct accum_out writes (ACT ln accums + DVE STT msum accums);
# cols 5-13: PE-colsum results copied from one [128,9] PSUM tile.
C_PO, C_POY, C_PG = 0, 1, 2   # box sigmoid(x) masked sums
C_X0, C_X1 = 3, 4             # sum x per sample
C_P0, C_P1 = 5, 6             # sum sigmoid(x) per sample
C_L0, C_L1 = 7, 8             # sum ln(1+u) per sample
C_LO, C_LG = 9, 10            # box ln masked sums
C_X1Y, C_XG, C_XOY = 11, 12, 13  # box x masked sums
OC = 14
PS0 = 0                       # every column is a PE-colsum into one PSUM tile


# --------------------------------------------------------------------------
# host-side connected components (scipy if present, numpy fallback)
# --------------------------------------------------------------------------

def _label_np(mask):
    """6-connectivity CC labeling, pure numpy (iterative min-propagation)."""
    lab = np.where(mask, np.arange(1, mask.size + 1, dtype=np.int64
                                   ).reshape(mask.shape), 0)
    while True:
        new = lab.copy()
        sl = new[1:, :, :]; np.minimum(sl, np.where(lab[:-1] > 0, lab[:-1], sl), out=sl)
        sl = new[:-1, :, :]; np.minimum(sl, np.where(lab[1:] > 0, lab[1:], sl), out=sl)
        sl = new[:, 1:, :]; np.minimum(sl, np.where(lab[:, :-1] > 0, lab[:, :-1], sl), out=sl)
        sl = new[:, :-1, :]; np.minimum(sl, np.where(lab[:, 1:] > 0, lab[:, 1:], sl), out=sl)
        sl = new[:, :, 1:]; np.minimum(sl, np.where(lab[:, :, :-1] > 0, lab[:, :, :-1], sl), out=sl)
        sl = new[:, :, :-1]; np.minimum(sl, np.where(lab[:, :, 1:] > 0, lab[:, :, 1:], sl), out=sl)
        new = np.where(mask, new, 0)
        if np.array_equal(new, lab):
            break
        lab = new
    uniq = np.unique(lab[lab > 0])
    remap = np.zeros(int(lab.max()) + 1, np.int64)
    remap[uniq] = np.arange(1, len(uniq) + 1)
    return remap[lab], len(uniq)


def _cc_label(mask):
    try:
        from scipy import ndimage as ndi
        st = ndi.generate_binary_structure(3, 1)
        lab, n = ndi.label(mask, structure=st)
        return lab.astype(np.int64), int(n)
    except Exception:
        return _label_np(mask)


CROP_MARGIN = 24   # predicted comps matched to a target stay well inside this


def _host_metadata(x, y):
    """Per-sample rank volumes t8/m8 and component counts.

    All labeling runs on a crop = target bounding box + CROP_MARGIN.  A
    predicted component can only be matched to a target if it intersects
    it, and matched components are small appendages of the targets, so
    anything outside the crop has t = m = 0.  The crop assumption is
    verified (no predicted foreground on the crop faces is labeled).
    """
    meta = []
    for b in range(B):
        tgt_full = y[b, 0] > 0.5
        pred_full = x[b, 0] >= 0.0
        if not tgt_full.any():
            meta.append(dict(t8=np.zeros((D, H, W), np.float32),
                             m8=np.zeros((D, H, W), np.float32), n_cc=0))
            continue
        idx = np.argwhere(tgt_full)
        lo = np.maximum(idx.min(axis=0) - CROP_MARGIN, 0)
        hi = np.minimum(idx.max(axis=0) + 1 + CROP_MARGIN, (D, H, W))
        sl = tuple(slice(int(a), int(c)) for a, c in zip(lo, hi))
        tgt = tgt_full[sl]
        pred = pred_full[sl]
        lin1 = (np.arange(N, dtype=np.int64).reshape(D, H, W)[sl] + 1)
        tlab, ntc = _cc_label(tgt)
        plab, npc = _cc_label(pred)
        # reference label value = max linear index + 1 within target comp
        tmax = np.zeros(ntc + 1, np.int64)
        np.maximum.at(tmax, tlab.ravel(), np.where(tgt, lin1, 0).ravel())
        tval = np.where(tgt, tmax[tlab], 0)
        # map each predicted comp to the max target label it overlaps
        pmax = np.zeros(npc + 1, np.int64)
        np.maximum.at(pmax, plab.ravel(), tval.ravel())
        mval = np.where(pred, pmax[plab], 0)
        # crop-validity: no matched predicted voxel may touch a crop face
        ok = True
        for ax in range(3):
            for face in (0, -1):
                f = [slice(None)] * 3
                f[ax] = face
                if (mval[tuple(f)] > 0).any():
                    ok = False
        # ranks: descending reference label order (top_k order)
        labels_desc = np.sort(np.unique(tval[tval > 0]))[::-1]
        n_cc = len(labels_desc)
        rank_of = np.zeros(int(tval.max()) + 1 if n_cc else 1, np.int64)
        for i, L in enumerate(labels_desc):
            rank_of[L] = i + 1
        t8 = np.zeros((D, H, W), np.float32)
        m8 = np.zeros((D, H, W), np.float32)
        t8[sl] = rank_of[tval]
        m8[sl] = rank_of[mval]
        meta.append(dict(t8=t8, m8=m8, n_cc=n_cc, crop_ok=ok))
    return meta


def _build_boxes(meta):
    """Cover the interesting voxels with <= NCORES boxes of BOX^3.

    Each connected cluster of the interesting set (target comp + its
    matched predicted comps) is covered by a grid of boxes over its bbox.
    Returns list of (sample, d0, h0, w0) and per-sample ownership arrays
    (box index owning each voxel, -1 if none).  Returns (None, None) when
    more than NCORES boxes would be needed (general fallback).
    """
    boxes = []
    owners = []
    for b in range(B):
        t8, m8 = meta[b]["t8"], meta[b]["m8"]
        interesting = (t8 > 0) | (m8 > 0)
        own = np.full((D, H, W), -1, np.int32)
        owners.append(own)
        if not interesting.any():
            continue
        clab, ncl = _cc_label(interesting)
        sample_boxes = []
        for ci in range(1, ncl + 1):
            idx = np.argwhere(clab == ci)
            lo, hi = idx.min(axis=0), idx.max(axis=0)  # inclusive
            starts_per_dim = []
            for ax in range(3):
                ext = int(hi[ax] - lo[ax] + 1)
                nb = (ext + BOX - 1) // BOX
                if nb == 1:
                    s0 = int(lo[ax]) - (BOX - ext) // 2
                    starts_per_dim.append([min(max(s0, 0), D - BOX)])
                else:
                    step = (ext - BOX) / (nb - 1)
                    starts_per_dim.append(
                        [min(max(int(lo[ax] + round(i * step)), 0), D - BOX)
                         for i in range(nb)])
            for sd in starts_per_dim[0]:
                for sh in starts_per_dim[1]:
                    for sw in starts_per_dim[2]:
                        bi = len(boxes)
                        if bi >= NCORES:
                            return None, None
                        boxes.append((b, sd, sh, sw))
                        sample_boxes.append((bi, ci, sd, sh, sw))
                        # interesting voxels of THIS cluster claim the box
                        sl = (slice(sd, sd + BOX), slice(sh, sh + BOX),
                              slice(sw, sw + BOX))
                        region = own[sl]
                        region[(clab[sl] == ci) & (region < 0)] = bi
        # background (non-interesting) voxels: first covering box wins
        for bi, ci, sd, sh, sw in sample_boxes:
            sl = (slice(sd, sd + BOX), slice(sh, sh + BOX),
                  slice(sw, sw + BOX))
            region = own[sl]
            region[region < 0] = bi
    for b in range(B):
        t8, m8 = meta[b]["t8"], meta[b]["m8"]
        if (((t8 > 0) | (m8 > 0)) & (owners[b] < 0)).any():
            return None, None
    return boxes, owners


def _box_ranks(meta, boxes, owners):
    """Per box: set of component ranks present among its owned voxels."""
    ranks = []
    for i, (bsmp, bd, bh, bw) in enumerate(boxes):
        sl = (slice(bd, bd + BOX), slice(bh, bh + BOX), slice(bw, bw + BOX))
        owned = owners[bsmp][sl] == i
        t = meta[bsmp]["t8"][sl][owned]
        m = meta[bsmp]["m8"][sl][owned]
        rs = set(np.unique(t[t > 0]).tolist()) | set(np.unique(m[m > 0]).tolist())
        ranks.append({int(r) for r in rs})
    return ranks


def _box_masks_counts(x, y, meta, boxes, owners):
    """Per box: the four bf16 0/1 mask planes + integer counts."""
    out = []
    for i, (bs, bd, bh, bw) in enumerate(boxes):
        sl = (slice(bd, bd + BOX), slice(bh, bh + BOX), slice(bw, bw + BOX))
        owned = owners[bs][sl] == i
        t8 = meta[bs]["t8"][sl]
        m8 = meta[bs]["m8"][sl]
        yb = y[bs, 0][sl] > 0.5
        o = owned
        oy = owned & yb
        o1y = owned & ~yb
        g = owned & (t8 == 0) & (m8 == 0)
        out.append(dict(
            o=o, oy=oy, o1y=o1y, g=g,
            cnt_o=float(o.sum()), cnt_oy=float(oy.sum()),
            cnt_g=float(g.sum()),
            xb=x[bs, 0][sl],
        ))
    return out


def _build_in_maps(x, y, meta, boxes, owners):
    """Per-core input tensors: gx [128, GTOT] bf16, mk [128, MCOLS] bf16."""
    bm = _box_masks_counts(x, y, meta, boxes, owners)
    in_maps = []
    for i in range(NCORES):
        d0 = i * SLAB
        gx = np.zeros((128, GTOT), np.float32)
        for s in range(B):
            gx[:, s * GS:(s + 1) * GS] = x[s, 0, d0:d0 + SLAB].reshape(128, GS)
        mk = np.zeros((128, MCOLS), np.float32)
        if i < len(boxes):
            gx[:, B * GS:] = bm[i]["xb"].reshape(128, BC)
            mk[:, 0 * BC:1 * BC] = bm[i]["o"].reshape(128, BC)
            mk[:, 1 * BC:2 * BC] = bm[i]["oy"].reshape(128, BC)
            mk[:, 2 * BC:3 * BC] = bm[i]["o1y"].reshape(128, BC)
            mk[:, 3 * BC:4 * BC] = bm[i]["g"].reshape(128, BC)
        in_maps.append(dict(
            gx=np.ascontiguousarray(gx.astype(BF16)),
            mk=np.ascontiguousarray(mk.astype(BF16)),
        ))
    return in_maps, bm


# --------------------------------------------------------------------------
# device kernel
# --------------------------------------------------------------------------

_BASS = {}


def _build_bass(fast=True):
    import concourse.bacc as bacc
    import concourse.tile as tile
    from concourse import mybir

    f32 = mybir.dt.float32
    bf16 = mybir.dt.bfloat16
    Alu = mybir.AluOpType
    Act = mybir.ActivationFunctionType

    HC = GS // 2  # 1024: DMA chunk cols within one sample slab

    nc = bacc.Bacc("TRN2", target_bir_lowering=False)
    gx = nc.dram_tensor("gx", [128, GTOT], bf16, kind="ExternalInput")
    mk = nc.dram_tensor("mk", [128, MCOLS], bf16, kind="ExternalInput")
    ot = nc.dram_tensor("ot", [128, OC], f32, kind="ExternalOutput")

    with tile.TileContext(nc) as tc:
        with tc.tile_pool(name="main", bufs=1) as pool, \
             tc.tile_pool(name="ps", bufs=1, space="PSUM") as ppool:

            acc = pool.tile([128, OC], f32, tag="acc")

            # ---- input tiles.  s0's first half is split 512/512 so ACT can
            # start ~0.9us earlier; the rest stream as 1024-col chunks ----
            QC = HC // 2
            xb = pool.tile([128, BC], bf16, tag="xb")
            mkt = pool.tile([128, MCOLS], bf16, tag="mkt")
            xs0 = [pool.tile([128, QC], bf16, tag=f"xs0{h}", name=f"xs0{h}")
                   for h in range(2)]
            xs0c = pool.tile([128, HC], bf16, tag="xs0c")
            xs1 = [pool.tile([128, HC], bf16, tag=f"xs1{h}", name=f"xs1{h}")
                   for h in range(2)]

            nc.sync.dma_start(xs0[0][:, :], gx[:, 0:QC])
            nc.sync.dma_start(xs0[1][:, :], gx[:, QC:HC])
            nc.sync.dma_start(xb[:, :], gx[:, B * GS:B * GS + BC])
            nc.sync.dma_start(xs0c[:, :], gx[:, HC:GS])
            nc.sync.dma_start(mkt[:, :], mk[:, :])
            nc.sync.dma_start(xs1[0][:, :], gx[:, GS:GS + HC])
            nc.sync.dma_start(xs1[1][:, :], gx[:, GS + HC:2 * GS])

            ones_bf = pool.tile([128, 1], bf16, tag="ones_bf")
            nc.gpsimd.memset(ones_bf[:, :], 1.0)
            ones_f = pool.tile([128, 1], f32, tag="ones_f")
            nc.gpsimd.memset(ones_f[:, :], 1.0)

            # every output column is a PE colsum chain into one PSUM tile
            psa = ppool.tile([128, OC], f32, tag="psa")

            def colsum(src, col, nch, onet, cols0=0):
                for j in range(nch):
                    nc.tensor.matmul(psa[:, col:col + 1],
                                     src[:, cols0 + j * 128:
                                         cols0 + (j + 1) * 128],
                                     onet[:, :], start=(j == 0),
                                     stop=(j == nch - 1))

            us = [pool.tile([128, GS], f32, tag=f"us{s}", name=f"us{s}")
                  for s in range(B)]
            ls = [pool.tile([128, GS], f32, tag=f"ls{s}", name=f"ls{s}")
                  for s in range(B)]
            ws = [pool.tile([128, GS], f32, tag=f"ws{s}", name=f"ws{s}")
                  for s in range(B)]
            ps_t = [pool.tile([128, GS], f32, tag=f"ps_t{s}", name=f"ps_t{s}")
                    for s in range(B)]
            ub = pool.tile([128, BC], f32, tag="ub")
            lb = pool.tile([128, BC], f32, tag="lb")
            wb = pool.tile([128, BC], f32, tag="wb")
            pb = pool.tile([128, BC], f32, tag="pb")

            # ---- ACT stream: all exps before the two ln passes, so the
            # DVE reciprocal chains (which need only u) run early and the
            # final P colsums are not pushed past the last ln ----
            nc.scalar.activation(us[0][:, 0:QC], xs0[0][:, :], Act.Exp,
                                 scale=-1.0)
            nc.scalar.activation(us[0][:, QC:HC], xs0[1][:, :], Act.Exp,
                                 scale=-1.0)
            nc.scalar.activation(ub[:, :], xb[:, :], Act.Exp, scale=-1.0)
            nc.scalar.activation(lb[:, :], ub[:, :], Act.Ln, bias=1.0)
            nc.scalar.activation(us[0][:, HC:GS], xs0c[:, :], Act.Exp,
                                 scale=-1.0)
            nc.scalar.activation(us[1][:, 0:HC], xs1[0][:, :], Act.Exp,
                                 scale=-1.0)
            nc.scalar.activation(us[1][:, HC:GS], xs1[1][:, :], Act.Exp,
                                 scale=-1.0)
            nc.scalar.activation(ls[0][:, :], us[0][:, :], Act.Ln, bias=1.0)
            nc.scalar.activation(ls[1][:, :], us[1][:, :], Act.Ln, bias=1.0)

            # ---- DVE stream: box w/p, then per-half w+reciprocal chains
            # (reciprocal is the only DVE division; divide/pow are invalid
            # ISA ops) -- subtile deps let each half start right after its
            # exp chunk lands ----
            nc.vector.tensor_scalar(wb[:, :], ub[:, :], 1.0, None, Alu.add)
            nc.vector.reciprocal(pb[:, :], wb[:, :])
            for s in range(B):
                for h in range(2):
                    lo, hi = h * HC, (h + 1) * HC
                    nc.vector.tensor_scalar(ws[s][:, lo:hi], us[s][:, lo:hi],
                                            1.0, None, Alu.add)
                    nc.vector.reciprocal(ps_t[s][:, lo:hi], ws[s][:, lo:hi])

            # ---- Pool stream: all eight box masked products ----
            MO, MOY, MO1Y, MG = (mkt[:, 0:BC], mkt[:, BC:2 * BC],
                                 mkt[:, 2 * BC:3 * BC], mkt[:, 3 * BC:4 * BC])
            box_sums = [(pb, MO, C_PO), (pb, MOY, C_POY), (pb, MG, C_PG),
                        (lb, MO, C_LO), (lb, MG, C_LG), (xb, MO1Y, C_X1Y),
                        (xb, MG, C_XG), (xb, MOY, C_XOY)]
            pscr = []
            for si, (field, mask, _col) in enumerate(box_sums):
                scr = pool.tile([128, BC], f32, tag=f"scrp{si}",
                                name=f"scrp{si}")
                nc.gpsimd.tensor_tensor(scr[:, :], field[:, :], mask, Alu.mult)
                pscr.append(scr)

            # ---- PE stream, ordered by expected data availability ----
            for s in range(B):
                src = [(xs0[0], 4), (xs0[1], 4), (xs0c, 8)] if s == 0 else \
                      [(xs1[0], 8), (xs1[1], 8)]
                first = True
                ktot = GS // 128
                k = 0
                for t, nch in src:
                    for j in range(nch):
                        nc.tensor.matmul(psa[:, C_X0 + s:C_X0 + s + 1],
                                         t[:, j * 128:(j + 1) * 128],
                                         ones_bf[:, :], start=(k == 0),
                                         stop=(k == ktot - 1))
                        k += 1
            for si, (_f, _m, col) in enumerate(box_sums):
                colsum(pscr[si], col, BC // 128, ones_f)
            colsum(ps_t[0], C_P0, GS // 128, ones_f)
            colsum(ls[0], C_L0, GS // 128, ones_f)
            colsum(ps_t[1], C_P1, GS // 128, ones_f)
            colsum(ls[1], C_L1, GS // 128, ones_f)

            # single psum -> acc copy (DVE; GPSIMD cannot access PSUM)
            nc.vector.tensor_scalar(acc[:, :], psa[:, :], 1.0, None, Alu.mult)

            nc.sync.dma_start(ot[:, :], acc[:, :])

    # all our activations (Exp/Ln) live in one table; hide the other tables
    # from the act-table-load pass so it emits a single load (keeps
    # act_func_set_id indices aligned with act_info.json by preserving order)
    import concourse.bacc as _bacc_mod
    _orig_tables = _bacc_mod.get_activation_tables
    _KEEP = "natural_log_exp_and_others"

    def _only_lnexp(arch):
        tabs = _orig_tables(arch)
        assert _KEEP in tabs
        return {name: (funcs if name == _KEEP else set())
                for name, funcs in tabs.items()}

    _bacc_mod.get_activation_tables = _only_lnexp
    try:
        nc.compile()
    finally:
        _bacc_mod.get_activation_tables = _orig_tables
    return nc


def _device_partials_np(in_maps):
    """Numpy mirror of the bass kernel, for pipeline validation."""
    outs = []
    for m in in_maps:
        gxv = np.asarray(m["gx"]).astype(np.float32)
        mkv = np.asarray(m["mk"]).astype(np.float32)
        acc = np.zeros((128, OC), np.float32)
        for s in range(B):
            xsv = gxv[:, s * GS:(s + 1) * GS]
            u = np.exp(-xsv).astype(np.float32)
            l = np.log1p(u).astype(np.float32)
            p = (1.0 / (u + 1.0)).astype(np.float32)
            acc[:, C_L0 + s] = l.sum(axis=1)
            acc[:, C_X0 + s] = xsv.sum(axis=1, dtype=np.float32)
            acc[:, C_P0 + s] = p.sum(axis=1)
        xbv = gxv[:, B * GS:]
        ub = np.exp(-xbv).astype(np.float32)
        lbv = np.log1p(ub).astype(np.float32)
        pbv = (1.0 / (ub + 1.0)).astype(np.float32)
        o, oy, o1y, g = (mkv[:, 0:BC], mkv[:, BC:2 * BC],
                         mkv[:, 2 * BC:3 * BC], mkv[:, 3 * BC:4 * BC])
        acc[:, C_PO] = (pbv * o).sum(axis=1)
        acc[:, C_POY] = (pbv * oy).sum(axis=1)
        acc[:, C_PG] = (pbv * g).sum(axis=1)
        acc[:, C_LO] = (lbv * o).sum(axis=1)
        acc[:, C_LG] = (lbv * g).sum(axis=1)
        acc[:, C_X1Y] = (xbv * o1y).sum(axis=1)
        acc[:, C_XG] = (xbv * g).sum(axis=1)
        acc[:, C_XOY] = (xbv * oy).sum(axis=1)
        outs.append(dict(ot=acc))
    return outs


_PJRT = {}


def _run_pjrt_cached(nc, in_maps):
    """run_bass_via_pjrt with the jitted executable cached across calls."""
    import jax
    from jax.experimental.shard_map import shard_map
    from jax.sharding import Mesh, PartitionSpec
    from concourse import bass2jax, mybir

    key = id(nc)
    if key not in _PJRT:
        bass2jax.install_neuronx_cc_hook()
        partition_name = (nc.partition_id_tensor.name
                          if nc.partition_id_tensor else None)
        in_names, out_names, out_avals, zero_shapes = [], [], [], []
        for alloc in nc.m.functions[0].allocations:
            if not isinstance(alloc, mybir.MemoryLocationSet):
                continue
            name = alloc.memorylocations[0].name
            if alloc.kind == "ExternalInput":
                if name != partition_name:
                    in_names.append(name)
            elif alloc.kind == "ExternalOutput":
                shape = tuple(alloc.tensor_shape)
                dtype = mybir.dt.np(alloc.dtype)
                out_names.append(name)
                out_avals.append(jax.core.ShapedArray(shape, dtype))
                zero_shapes.append((shape, dtype))
        n_params = len(in_names)
        n_outs = len(out_avals)
        all_in_names = list(in_names) + list(out_names)
        if partition_name is not None:
            all_in_names.append(partition_name)

        def _body(*args):
            operands = list(args)
            if partition_name is not None:
                operands.append(bass2jax.partition_id_tensor())
            outs = bass2jax._bass_exec_p.bind(
                *operands,
                out_avals=tuple(out_avals),
                in_names=tuple(all_in_names),
                out_names=tuple(out_names),
                lowering_input_output_aliases=(),
                sim_require_finite=True,
                sim_require_nnan=True,
                nc=nc,
            )
            return tuple(outs)

        devices = jax.devices()[:NCORES]
        assert len(devices) == NCORES
        mesh = Mesh(np.asarray(devices), ("core",))
        donate = tuple(range(n_params, n_params + n_outs))
        sharded = jax.jit(
            shard_map(_body, mesh=mesh,
                      in_specs=(PartitionSpec("core"),) * (n_params + n_outs),
                      out_specs=(PartitionSpec("core"),) * n_outs,
                      check_rep=False),
            donate_argnums=donate, keep_unused=True)
        _PJRT[key] = (sharded, in_names, out_names, out_avals, zero_shapes)

    sharded, in_names, out_names, out_avals, zero_shapes = _PJRT[key]
    concat_in = [
        np.concatenate([np.asarray(m[name]) for m in in_maps], axis=0)
        for name in in_names
    ]
    concat_zeros = [
        np.zeros((NCORES * s[0], *s[1:]), dt) for s, dt in zero_shapes
    ]
    out_arrs = sharded(*concat_in, *concat_zeros)
    return [
        {name: np.asarray(out_arrs[i]).reshape(NCORES, *out_avals[i].shape)[c]
         for i, name in enumerate(out_names)}
        for c in range(NCORES)
    ]


def _device_partials(in_maps, fast=True):
    if os.environ.get("BLOB_KERNEL_NP"):
        return _device_partials_np(in_maps)
    try:
        if True not in _BASS:
            _BASS[True] = _build_bass(True)
            _BASS[False] = _BASS[True]
        return _run_pjrt_cached(_BASS[True], in_maps)
    except Exception:
        if os.environ.get("BLOB_NO_FALLBACK"):
            raise
        import traceback
        traceback.print_exc()
        print("blob kernel: device path failed; using numpy fallback",
              flush=True)
        return _device_partials_np(in_maps)


# --------------------------------------------------------------------------
# general fallback: full-volume numpy evaluation of the reference loss
# --------------------------------------------------------------------------

def _loss_general_np(x, y, meta):
    x = x.astype(np.float64)
    y = y.astype(np.float64)

    def dc_bce(xv, yv):
        # mean BCEWithLogits + batch dice over the given [b,1,D,H,W] arrays
        bce = np.mean(np.logaddexp(0.0, xv) - xv * yv)
        p = 1.0 / (1.0 + np.exp(-xv))
        I = (p * yv).sum()
        P = p.sum()
        G = yv.sum()
        dc = (2.0 * I + SMOOTH) / max(P + G + SMOOTH, 1e-8)
        return bce - dc

    global_loss = dc_bce(x, y)
    total_contrib, total_count = 0.0, 0.0
    for s in range(B):
        t8, m8, n_cc = meta[s]["t8"], meta[s]["m8"], meta[s]["n_cc"]
        xi, yi = x[s:s + 1], y[s:s + 1]
        if n_cc > 1:
            for c in range(1, n_cc + 1):
                keep = (((t8 == 0) | (t8 == c)) & ((m8 == 0) | (m8 == c))
                        ).astype(np.float64)[None, None]
                total_contrib += dc_bce(xi * keep, yi * keep)
            total_count += n_cc
        else:
            total_contrib += dc_bce(xi, yi)
            total_count += 1
    blob = total_contrib / max(total_count, 1.0)
    return 0.3 * global_loss + 0.7 * blob


# --------------------------------------------------------------------------
# public entry
# --------------------------------------------------------------------------

def kernel(net_output, target):
    x = np.ascontiguousarray(np.asarray(net_output, dtype=np.float32))
    y = np.ascontiguousarray(np.asarray(target, dtype=np.float32))
    assert x.shape == (B, 1, D, H, W) and y.shape == x.shape

    meta = _host_metadata(x, y)
    boxes, owners = _build_boxes(meta)
    general = (
        boxes is None
        or any(not m.get("crop_ok", True) for m in meta)
        or any(m["n_cc"] > K_DEV for m in meta)
        or os.environ.get("BLOB_FORCE_GENERAL")
    )
    if not general:
        ranks = _box_ranks(meta, boxes, owners)
        general = any(len(r) > 1 for r in ranks)
    if general:
        return np.asarray(_loss_general_np(x, y, meta), dtype=np.float32)

    in_maps, bm = _build_in_maps(x, y, meta, boxes, owners)
    results = _device_partials(in_maps, True)

    # ------------------------ host assembly (O(1)) ------------------------
    cols = np.zeros((NCORES, OC), np.float64)
    for i, r in enumerate(results):
        cols[i] = np.asarray(r["ot"], np.float64).sum(axis=0)

    glob = []
    for s in range(B):
        A = cols[:, C_L0 + s].sum() + cols[:, C_X0 + s].sum()
        P = cols[:, C_P0 + s].sum()
        glob.append(dict(A=A, P=P))

    XY = np.zeros(B); I = np.zeros(B); G = np.zeros(B)
    names = ["f1", "p", "py", "y", "cnt"]
    corr = [[dict(f1=0.0, p=0.0, py=0.0, y=0.0, cnt=0.0)
             for _ in range(K_DEV + 1)] for _ in range(B)]
    for i, (bs, bd, bh, bw) in enumerate(boxes):
        c = cols[i]
        cnt_o, cnt_oy, cnt_g = bm[i]["cnt_o"], bm[i]["cnt_oy"], bm[i]["cnt_g"]
        own = dict(f1=c[C_LO] + c[C_X1Y], p=c[C_PO],
                   py=c[C_POY], y=cnt_oy, cnt=cnt_o)
        bg = dict(f1=c[C_LG] + c[C_XG], p=c[C_PG],
                  py=0.0, y=0.0, cnt=cnt_g)
        XY[bs] += c[C_XOY]
        I[bs] += own["py"]
        G[bs] += cnt_oy
        rset = ranks[i]
        for lab in range(1, K_DEV + 1):
            kp = own if (rset and lab in rset) else bg
            for nm in names:
                corr[bs][lab][nm] += kp[nm] - own[nm]

    total_contrib = 0.0
    total_count = 0.0
    for s in range(B):
        n_cc = meta[s]["n_cc"]
        gg = dict(f1=glob[s]["A"] - XY[s], p=glob[s]["P"], py=I[s], y=G[s],
                  cnt=float(N))
        if n_cc > 1:
            for lab in range(1, n_cc + 1):
                Sf = {nm: gg[nm] + corr[s][lab][nm] for nm in names}
                nk = Sf["cnt"]
                bce = (Sf["f1"] + LOG2 * (N - nk)) / N
                Pc = Sf["p"] + 0.5 * (N - nk)
                dc = (2.0 * Sf["py"] + SMOOTH) / max(Pc + Sf["y"] + SMOOTH, 1e-8)
                total_contrib += bce - dc
            total_count += n_cc
        else:
            bce = gg["f1"] / N
            dc = (2.0 * gg["py"] + SMOOTH) / max(gg["p"] + gg["y"] + SMOOTH, 1e-8)
            total_contrib += bce - dc
            total_count += 1

    f1b = sum(glob[s]["A"] - XY[s] for s in range(B))
    bce_g = f1b / (B * N)
    Ib = I.sum(); Pb = sum(g["P"] for g in glob); Gb = G.sum()
    dc_g = (2.0 * Ib + SMOOTH) / max(Pb + Gb + SMOOTH, 1e-8)
    global_loss = bce_g - dc_g

    blob = total_contrib / max(total_count, 1.0)
    out = 0.3 * global_loss + 0.7 * blob
    return np.asarray(out, dtype=np.float32)
